# revision 20
# baseline (speedup 1.0000x reference)
"""Multi-head attention layer (B=2, L=2048, H=1024, 16 heads) on 8 TRN2
NeuronCores -- head-tensor-parallel, fp8-DoubleRow version.

Sharding: core c -> (batch b = c//4, head group g = c%4, i.e. heads
4g..4g+3 = hidden dims [256g, 256g+256)).  Each core computes Q/K/V
projections only for its own 4 heads over the full sequence (zero
duplication, vs 4x duplicated K/V in the query-sharded layout), runs
attention for those heads over all 2048 queries, then the four cores of a
batch exchange attention outputs with four 4-rank AllGathers (bf16,
128KB each, one per head-pair x q-half, on TOPSP+SDMA so all but the
last overlap compute).  After the exchange, core g runs the output
projection + residual + LayerNorm for q-rows [512g, 512g+512) of its
batch.  The gathered buffer is read with a partition-id-indexed DMA
(SPMD-safe dynamic offset); post-AG loads ride the gpsimd queue so an
in-flight collective never head-of-line-blocks the sync DMA queue.

Precision: Q/K/V projections and P@V run fp8e4 DoubleRow (2 fp8 rows per
PE cell -> half the matmul passes).  Weights are scaled x16 on the host
to clear e4m3's subnormal zone; the scale is folded into the exp
(0.125/256) and cancelled by LayerNorm's scale invariance (residual is
pre-scaled x16 host-side).  Scores (dk=64 contraction) and the output
projection stay bf16; LayerNorm runs bf16 with the y output returned as
bf16 and widened on the host.  Final rel err ~6e-3 (budget 2e-2).

Pipeline: per (head, q-half of 1024) unit, the kt-loop runs scores(kt)
-> exp(kt) on ScalarE -> P@V(kt-pair, delayed 4 kt), with scores psum
2-3 deep so ScalarE (the ~139us/core exp floor) and the PE overlap.
Q/K/V projection chunks are hand-interleaved into the first units' kt
loops to fill the PE slack.  V carries a ones column (padded to 128
stationary columns for DoubleRow) so the softmax denominator falls out
of the P@V matmul; 1/Z via reciprocal_approx_fast (SBUF-staged -- the
custom DVE op reads garbage directly from PSUM) and a DRAM broadcast
round-trip.  P@V psum is drained to SBUF immediately so the next unit
can reuse the bank while normalization runs off the critical path.
"""

import sys

if "/opt/trn_rl_repo" not in sys.path:
    sys.path.insert(0, "/opt/trn_rl_repo")

import ml_dtypes
import numpy as np

import concourse.bass as bass
import concourse.tile as tile
from concourse import bacc, mybir
from concourse.bass_utils import run_bass_kernel_spmd

F32 = mybir.dt.float32
BF16 = mybir.dt.bfloat16
FP8 = mybir.dt.float8e4
AF = mybir.ActivationFunctionType
BF = ml_dtypes.bfloat16
E4 = ml_dtypes.float8_e4m3
DR = mybir.MatmulPerfMode.DoubleRow
WS = 16.0  # fp8 weight scale: keeps uniform(-1/32,1/32) weights out of
           # e4m3's subnormal zone; folded into exp scale / LN invariance

B = 2
L = 2048
H = 1024
NH = 16
DK = 64
P = 128
HT = H // P       # 8 contraction tiles over hidden dim
LT = L // P       # 16 key tiles over sequence
DT = 2            # 256 head-dims per core = 2 partition tiles
DC = 256          # head dims per core
QR = 512          # output q-rows per core
GROUPS = [[0, 1, 2, 3], [4, 5, 6, 7]]


def build_module(debug: bool = False) -> bass.Bass:
    nc = bacc.Bacc("TRN2", target_bir_lowering=False, num_devices=8)

    t = {
        # host pre-tiled to [partition, ...contiguous] for fast DMA
        "xT": nc.dram_tensor("xT", [P, 4, HT, 512], FP8, kind="ExternalInput"),
        "wqT": nc.dram_tensor("wqT", [P, HT, 4, P], FP8, kind="ExternalInput"),
        "wkT": nc.dram_tensor("wkT", [P, HT, 4, P], FP8, kind="ExternalInput"),
        "wvT": nc.dram_tensor("wvT", [P, HT, DC], FP8, kind="ExternalInput"),
        "woT": nc.dram_tensor("woT", [P, HT, H], BF16, kind="ExternalInput"),
        "bqT": nc.dram_tensor("bqT", [P, 4], F32, kind="ExternalInput"),
        "bkT": nc.dram_tensor("bkT", [P, 4], F32, kind="ExternalInput"),
        "bvb": nc.dram_tensor("bvb", [DC], F32, kind="ExternalInput"),
        "gamma": nc.dram_tensor("gamma", [H], BF16, kind="ExternalInput"),
        "beta": nc.dram_tensor("beta", [H], BF16, kind="ExternalInput"),
        "xq": nc.dram_tensor("xq", [QR, H], F32, kind="ExternalInput"),
        "y": nc.dram_tensor("y", [QR, H], BF16, kind="ExternalOutput"),
    }
    # AllGather bounce buffers, one 4-rank collective per (head-pair,
    # q-half): in [2 qblocks, 128 dims, 512]; gathered rank-major into rows
    # [8*qh + 2*j + qb] of a [16, 128, 512] view for dynamic-index reads.
    agin = [
        [nc.dram_tensor(f"agin{p}{qh}", [2, P, 512], BF16) for qh in range(2)]
        for p in range(2)
    ]
    agout = [nc.dram_tensor(f"agout{p}", [16, P, 512], BF16) for p in range(2)]

    dbg = None
    if debug:
        dbg = {
            "qT": nc.dram_tensor("d_qT", [P, DT, L], BF16, kind="ExternalOutput"),
            "kT": nc.dram_tensor("d_kT", [P, DT, L], BF16, kind="ExternalOutput"),
            "v": nc.dram_tensor("d_v", [P, LT, 4, DK + 1], BF16, kind="ExternalOutput"),
            "agin0": nc.dram_tensor("d_agin0", [2, 2, P, 512], BF16, kind="ExternalOutput"),
            "agout0": nc.dram_tensor("d_agout0", [16, P, 512], BF16, kind="ExternalOutput"),
            "oAll": nc.dram_tensor("d_oAll", [P, HT, 512], BF16, kind="ExternalOutput"),
        }

    with tile.TileContext(nc) as tc:
        _build(tc, nc, t, agin, agout, dbg)
    nc.compile()
    return nc


def _build(tc, nc, t, agin, agout, dbg=None):
    def bcast(dram, parts=P):
        ap = dram[:]
        return bass.AP(tensor=ap.tensor, offset=ap.offset, ap=[[0, parts], *ap.ap])

    with (
        tc.tile_pool(name="const", bufs=1) as const,
        tc.tile_pool(name="big", bufs=1) as big,
    ):
        # --- constants -------------------------------------------------
        bqT_sb = const.tile([P, 4], F32)
        bkT_sb = const.tile([P, 4], F32)
        bvB = const.tile([P, DC], F32)
        gB = const.tile([P, H], BF16)
        btB = const.tile([P, H], BF16)
        eps_sb = const.tile([P, 1], F32)
        nc.vector.memset(eps_sb, 1e-5)

        # --- persistent tensors (DMA order: first-needed first; q-half 0
        # of x goes down both hwdge queues in parallel) ------------------
        wq_sb = big.tile([P, HT, 4, P], FP8)
        wk_sb = big.tile([P, HT, 4, P], FP8)
        wv_sb = big.tile([P, HT, DC], FP8)
        xT_sb = big.tile([P, HT, L], FP8)
        nc.sync.dma_start(out=wq_sb, in_=t["wqT"][:])
        nc.scalar.dma_start(
            out=xT_sb[:, :, 512:1024], in_=t["xT"][:, 1, :, :]
        )
        nc.sync.dma_start(out=wk_sb, in_=t["wkT"][:])
        nc.sync.dma_start(
            out=xT_sb[:, :, 0:512], in_=t["xT"][:, 0, :, :]
        )
        nc.sync.dma_start(out=wv_sb, in_=t["wvT"][:])
        nc.sync.dma_start(out=bqT_sb, in_=t["bqT"][:])
        nc.sync.dma_start(out=bkT_sb, in_=t["bkT"][:])
        nc.sync.dma_start(out=bvB, in_=bcast(t["bvb"]))
        for qc in range(2, 4):
            nc.sync.dma_start(
                out=xT_sb[:, :, qc * 512 : (qc + 1) * 512],
                in_=t["xT"][:, qc, :, :],
            )
        woT_sb = big.tile([P, HT, H], BF16)
        nc.sync.dma_start(out=woT_sb, in_=t["woT"][:])
        nc.sync.dma_start(out=gB, in_=bcast(t["gamma"]))
        nc.sync.dma_start(out=btB, in_=bcast(t["beta"]))

        qT_sb = big.tile([P, 4, L], BF16)
        kT_sb = big.tile([P, 4, L], BF16)
        # V padded to 128 stationary columns for DoubleRow (col_grp must be
        # 0xf): cols 0-63 = V dims, col 64 = ones (softmax Z), 65-127 zero.
        v_sb = big.tile([P, LT, 4, P], FP8)
        nc.vector.memset(v_sb[:, :, :, DK:], 0.0)
        nc.vector.memset(v_sb[:, :, :, DK : DK + 1], 1.0)
        oAll_sb = big.tile([P, HT, 512], BF16)

        pools = {}

        # --- projection chunks (fp8 DoubleRow: 4 ht-pair passes) --------
        def v_chunk(kt):
            ps = pools["psP"].tile([P, 512], F32, tag="pp")
            for tp in range(4):
                nc.tensor.matmul(
                    ps[:, 0:DC],
                    lhsT=xT_sb[:, 2 * tp : 2 * tp + 2, kt * P : (kt + 1) * P],
                    rhs=wv_sb[:, 2 * tp : 2 * tp + 2, :],
                    start=(tp == 0),
                    stop=(tp == 3),
                    perf_mode=DR,
                )
            nc.vector.tensor_add(
                out=v_sb[:, kt, :, 0:DK],
                in0=ps[:, 0:DC].rearrange("p (hh d) -> p hh d", d=DK),
                in1=bvB[:].rearrange("p (hh d) -> p hh d", d=DK),
            )

        def qk_chunk(w_sb, o_sb, b_sb, h, qc):
            # stationary columns hold the head's 64 dims TWICE, so qT/kT end
            # up duplicated across both partition halves -> score matmuls for
            # two key tiles can run in concurrent PE row groups
            ps = pools["psP"].tile([P, 512], F32, tag="pp")
            for tp in range(4):
                nc.tensor.matmul(
                    ps,
                    lhsT=w_sb[:, 2 * tp : 2 * tp + 2, h, :],
                    rhs=xT_sb[:, 2 * tp : 2 * tp + 2, qc * 512 : (qc + 1) * 512],
                    start=(tp == 0),
                    stop=(tp == 3),
                    perf_mode=DR,
                )
            nc.vector.tensor_scalar_add(
                out=o_sb[:, h, qc * 512 : (qc + 1) * 512],
                in0=ps,
                scalar1=b_sb[:, h : h + 1],
            )

        def q_chunk(h, qc):
            qk_chunk(wq_sb, qT_sb, bqT_sb, h, qc)

        def k_chunk(h, qc):
            qk_chunk(wk_sb, kT_sb, bkT_sb, h, qc)

        # --- one attention unit: head h, q-half qh (1024 queries) ------
        # P@V runs fp8 DoubleRow over kt-pairs: stationary = V for two key
        # tiles [128, 2, 128], moving = the pair's exp tile [128, 2, 512].
        def pv_mm(pv, h, g, ptp):
            for u in range(2):
                nc.tensor.matmul(
                    pv[:, u, :],
                    lhsT=v_sb[:, 2 * g : 2 * g + 2, h, :],
                    rhs=ptp[:, :, u, :],
                    start=(g == 0),
                    stop=(g == LT // 2 - 1),
                    perf_mode=DR,
                )

        PVD = 4  # P@V (per kt-pair) trails exp so PE never waits on a fresh exp

        def attn_unit(h, qh, fillers, s3=False, last=False):
            q0 = qh * 1024
            po = DK * (h % 2)  # dim offset within the head-pair's agin rows
            pv = pools["psV"].tile([P, 2, 512], F32, tag="pv")
            ptps = [None] * (LT // 2)

            def s_tile(kt):
                sp = pools["psS2"] if (s3 and kt % 3 == 2) else pools["psS"]
                st = sp.tile([P, 2, 512], F32, tag="s")
                return st

            for g in range(LT // 2):
                ktA, ktB = 2 * g, 2 * g + 1
                SA = s_tile(ktA)
                SB = s_tile(ktB)
                # qT/kT hold each head's 64 dims in BOTH partition halves, so
                # the ktA matmul (array rows 0-63) and the ktB matmul (rows
                # 64-127, from base_partition 64) execute concurrently in
                # disjoint PE row groups -> 2x scores throughput.
                for u in range(2):
                    nc.tensor.matmul(
                        SA[:, u, :],
                        lhsT=kT_sb[0:DK, h, ktA * P : (ktA + 1) * P],
                        rhs=qT_sb[0:DK, h, q0 + u * 512 : q0 + (u + 1) * 512],
                        start=True,
                        stop=True,
                    )
                    nc.tensor.matmul(
                        SB[:, u, :],
                        lhsT=kT_sb[DK:P, h, ktB * P : (ktB + 1) * P],
                        rhs=qT_sb[DK:P, h, q0 + u * 512 : q0 + (u + 1) * 512],
                        start=True,
                        stop=True,
                    )
                ptp_t = pools["pt"].tile([P, 2, 2, 512], FP8, tag="pt")
                ptps[g] = ptp_t
                # scores are 256x true (16x-scaled Wq and Wk): fold into exp
                nc.scalar.activation(
                    out=ptp_t[:, 0, :, :], in_=SA, func=AF.Exp,
                    scale=0.125 / (WS * WS),
                )
                nc.scalar.activation(
                    out=ptp_t[:, 1, :, :], in_=SB, func=AF.Exp,
                    scale=0.125 / (WS * WS),
                )
                for _ in range(2):
                    if fillers:
                        fillers.pop(0)()
                if g >= 2:
                    pv_mm(pv, h, g - 2, ptps[g - 2])
            for g in range(LT // 2 - 2, LT // 2):
                pv_mm(pv, h, g, ptps[g])
            # Drain PV psum right away (raw copies) so the next unit's P@V
            # can reuse the bank; normalize runs off the critical path.
            zc = pools["zr"].tile([1, 2, 512], F32, tag="zc")
            nc.vector.tensor_copy(zc, pv[DK : DK + 1, :, :])
            praw = pools["praw"].tile([DK, 2, 512], BF16, tag="praw")
            nc.vector.tensor_copy(praw, pv[0:DK, :, :])
            zr = pools["zr"].tile([1, 2, 512], F32, tag="zr")
            nc.vector.reciprocal_approx_fast(out=zr, in_=zc)
            zd = pools["zd"].tile([1024], F32, tag="zd")
            nc.sync.dma_start(out=zd, in_=zr[:].rearrange("p u q -> p (u q)"))
            zb = pools["zb"].tile([DK, 1024], F32, tag="zb")
            nc.sync.dma_start(out=zb, in_=bcast(zd, parts=DK))
            ot = pools["ot"].tile([DK, 2, 512], BF16, tag="ot")
            nc.vector.tensor_mul(
                out=ot,
                in0=praw,
                in1=zb[:].rearrange("p (u q) -> p u q", u=2),
            )
            pr = h // 2
            eng = nc.scalar if last else nc.sync
            eng.dma_start(
                out=agin[pr][qh][:, po : po + DK, :].rearrange("u d q -> d u q"),
                in_=ot,
            )

        # ===== emission ================================================
        gsym = nc.gpsimd.partition_id() % 4

        def emit_ag(p, qh):
            nc.gpsimd.collective_compute(
                "AllGather",
                mybir.AluOpType.bypass,
                replica_groups=GROUPS,
                ins=[agin[p][qh][:].opt()],
                outs=[agout[p][8 * qh : 8 * (qh + 1)].opt()],
            )

        def emit_loads(p):
            # row = 8*qh + 2*j + qb with (qh, qb) = (g//2, g%2); on the
            # gpsimd queue so waiting on the AG never blocks the sync queue.
            for j in range(4):
                idx = (gsym // 2) * 8 + 2 * j + (gsym % 2)
                nc.gpsimd.dma_start(
                    out=oAll_sb[:, 2 * j + p, :], in_=agout[p][idx]
                )

        with (
            tc.tile_pool(name="psS", bufs=2, space="PSUM") as psS,
            tc.tile_pool(name="psV", bufs=1, space="PSUM") as psV,
            tc.tile_pool(name="pt", bufs=6) as ptp,
            tc.tile_pool(name="praw", bufs=3) as prawp,
            tc.tile_pool(name="ot", bufs=3) as otp,
            tc.tile_pool(name="zr", bufs=2) as zrp,
            tc.tile_pool(name="zb", bufs=2) as zbp,
            tc.tile_pool(name="zd", bufs=3, space="DRAM") as zdp,
        ):
            pools.update(psS=psS, psV=psV, pt=ptp, praw=prawp,
                         ot=otp, zr=zrp, zb=zbp, zd=zdp)

            with tc.tile_pool(name="psP", bufs=2, space="PSUM") as psP:
                pools["psP"] = psP
                # small HAM pre-warm: ~15 dummy matmuls (~= the input
                # DMA wait) so the PE hits K=8/8 before stage 0
                dw_sb = big.tile([P, 512], BF16)
                nc.vector.memset(dw_sb, 0.01)
                pw = psP.tile([P, 512], F32, tag="pp")
                for _ in range(15):
                    nc.tensor.matmul(
                        pw, lhsT=dw_sb[:, 0:P], rhs=dw_sb, start=True,
                        stop=True,
                    )
                # stage 0: minimum prereqs for units (0,0) and (1,0)
                q_chunk(0, 0)
                q_chunk(0, 1)
                k_chunk(0, 0)
                q_chunk(1, 0)
                q_chunk(1, 1)
                for kt in range(4):
                    v_chunk(kt)

                def Q(h, qc):
                    return lambda: q_chunk(h, qc)

                def K(h, qc):
                    return lambda: k_chunk(h, qc)

                fillers = (
                    [K(0, 1), K(1, 0), K(0, 2), K(0, 3)]
                    + [(lambda kt=kt: v_chunk(kt)) for kt in range(4, LT)]
                )
                fillers2 = [
                    K(1, 1), K(1, 2), K(1, 3), Q(0, 2), Q(0, 3),
                    Q(1, 2), Q(1, 3), Q(2, 0), Q(2, 1),
                    K(2, 0), K(2, 1), K(2, 2), K(2, 3),
                    Q(3, 0), Q(3, 1), K(3, 0), K(3, 1), K(3, 2), K(3, 3),
                    Q(2, 2), Q(2, 3), Q(3, 2), Q(3, 3),
                ]

                attn_unit(0, 0, fillers)
                attn_unit(1, 0, fillers2)
                while fillers2:  # all projections done before psP closes
                    fillers2.pop(0)()
                emit_ag(0, 0)

            # filler psum closed -> 2 banks free for the 3rd scores buffer
            with tc.tile_pool(name="psS2", bufs=1, space="PSUM") as psS2:
                pools["psS2"] = psS2
                attn_unit(0, 1, None, s3=True)
                attn_unit(1, 1, None, s3=True)
                emit_ag(0, 1)
                emit_loads(0)
                attn_unit(2, 0, None, s3=True)
                attn_unit(3, 0, None, s3=True)
                emit_ag(1, 0)
                attn_unit(2, 1, None, s3=True)
                attn_unit(3, 1, None, s3=True, last=True)
                emit_ag(1, 1)
                emit_loads(1)

        if dbg is not None:
            nc.sync.dma_start(out=dbg["qT"][:], in_=qT_sb)
            nc.sync.dma_start(out=dbg["kT"][:], in_=kT_sb)
            nc.sync.dma_start(out=dbg["v"][:], in_=v_sb)
            nc.sync.dma_start(out=dbg["agin0"][0], in_=agin[0][0][:])
            nc.sync.dma_start(out=dbg["agin0"][1], in_=agin[0][1][:])
            nc.sync.dma_start(out=dbg["agout0"][:], in_=agout[0][:])
            nc.sync.dma_start(out=dbg["oAll"][:], in_=oAll_sb)

        # ===== output projection + residual + LayerNorm ================
        with (
            tc.tile_pool(name="psY", bufs=4, space="PSUM") as psY,
            tc.tile_pool(name="yp", bufs=4) as yp,
            tc.tile_pool(name="ln", bufs=4) as lnp,
        ):
            NQT = QR // P
            ys = []
            for qt in range(NQT):  # pass A: even head-pair dims (during AG#2)
                psy = psY.tile([P, 2, 512], F32, tag="psy")
                ys.append(psy)
                for ht in (0, 2, 4, 6):
                    for u in range(2):
                        nc.tensor.matmul(
                            psy[:, u, :],
                            lhsT=oAll_sb[:, ht, qt * P : (qt + 1) * P],
                            rhs=woT_sb[:, ht, u * 512 : (u + 1) * 512],
                            start=(ht == 0),
                            stop=False,
                        )
            xqs = []
            for qt in range(NQT):  # residual rows: off the critical chain
                xq_t = yp.tile([P, H], F32, tag="xq")
                nc.sync.dma_start(out=xq_t, in_=t["xq"][qt * P : (qt + 1) * P, :])
                xqs.append(xq_t)
            for qt in range(NQT):  # pass B + LayerNorm + store
                psy = ys[qt]
                for ht in (1, 3, 5, 7):
                    for u in range(2):
                        nc.tensor.matmul(
                            psy[:, u, :],
                            lhsT=oAll_sb[:, ht, qt * P : (qt + 1) * P],
                            rhs=woT_sb[:, ht, u * 512 : (u + 1) * 512],
                            start=False,
                            stop=(ht == 7),
                        )
                y_t = yp.tile([P, H], BF16, tag="y")
                nc.vector.tensor_add(
                    out=y_t,
                    in0=psy[:].rearrange("p u q -> p (u q)"),
                    in1=xqs[qt],
                )
                stats = lnp.tile([P, 2, 6], F32, tag="st")
                nc.vector.bn_stats(out=stats[:, 0, :], in_=y_t[:, 0:512])
                nc.vector.bn_stats(out=stats[:, 1, :], in_=y_t[:, 512:1024])
                mv = lnp.tile([P, 2], F32, tag="mv")
                nc.vector.bn_aggr(out=mv, in_=stats)
                rstd = lnp.tile([P, 1], F32, tag="rs")
                nc.scalar.activation(
                    out=rstd, in_=mv[:, 1:2], func=AF.Sqrt, bias=eps_sb, scale=1.0
                )
                nc.vector.reciprocal(out=rstd, in_=rstd)
                nc.vector.tensor_scalar(
                    out=y_t,
                    in0=y_t,
                    scalar1=mv[:, 0:1],
                    scalar2=rstd,
                    op0=mybir.AluOpType.subtract,
                    op1=mybir.AluOpType.mult,
                )
                y_g = yp.tile([P, H], BF16, tag="yg")
                nc.gpsimd.tensor_mul(out=y_g, in0=y_t, in1=gB)
                y_o = yp.tile([P, H], BF16, tag="yo")
                nc.gpsimd.tensor_add(out=y_o, in0=y_g, in1=btB)
                nc.sync.dma_start(out=t["y"][qt * P : (qt + 1) * P, :], in_=y_o)


_BUILT = None


def _get_nc():
    global _BUILT
    if _BUILT is None:
        _BUILT = build_module()
    return _BUILT


def make_in_maps(x, Wq, bq, Wk, bk, Wv, bv, Wo, bo, ln_gamma, ln_beta) -> list[dict]:
    f32 = lambda a: np.ascontiguousarray(np.asarray(a, dtype=np.float32))
    bf = lambda a: np.ascontiguousarray(np.asarray(a, dtype=np.float32).T.astype(BF))
    x = f32(x)
    tr = lambda a: np.ascontiguousarray(np.asarray(a, dtype=np.float32).T)
    wqT, wkT, wvT, woT = tr(Wq), tr(Wk), tr(Wv), tr(Wo)
    bq, bk, bv = f32(bq), f32(bk), f32(bv)
    bo = f32(bo)
    ws = np.float32(WS)

    def ptile(a, dt):  # [1024, D] -> [128, 8, D] partition-contiguous
        return np.ascontiguousarray(
            a.reshape(HT, P, a.shape[1]).transpose(1, 0, 2).astype(dt)
        )

    xTs = [
        np.ascontiguousarray(
            tr(x[b]).reshape(HT, P, 4, 512).transpose(1, 2, 0, 3).astype(E4)
        )
        for b in range(B)
    ]
    shared = {
        "woT": ptile(woT, BF),
        "gamma": np.ascontiguousarray(f32(ln_gamma).astype(BF)),
        "beta": np.ascontiguousarray(f32(ln_beta).astype(BF)),
    }
    def dup_w(wT, g):
        # [1024, 4 heads, 128]: each head's 64 columns duplicated so the
        # projection writes the head's dims into both partition halves
        cols = []
        for h in range(4):
            c = ws * wT[:, DC * g + DK * h : DC * g + DK * (h + 1)]
            cols.append(np.concatenate([c, c], axis=1))
        a = np.stack(cols, axis=1)
        return np.ascontiguousarray(
            a.reshape(HT, P, 4, P).transpose(1, 0, 2, 3).astype(E4)
        )

    def dup_b(bv_, g):
        return np.ascontiguousarray(
            np.stack(
                [
                    np.tile(ws * bv_[DC * g + DK * h : DC * g + DK * (h + 1)], 2)
                    for h in range(4)
                ],
                axis=1,
            ).astype(np.float32)
        )

    in_maps = []
    for c in range(8):
        b, g = divmod(c, 4)
        dims = slice(DC * g, DC * (g + 1))
        rows = slice(QR * g, QR * (g + 1))
        in_maps.append(
            {
                "xT": xTs[b],
                "wqT": dup_w(wqT, g),
                "wkT": dup_w(wkT, g),
                "wvT": ptile(ws * wvT[:, dims], E4),
                "bqT": dup_b(bq, g),
                "bkT": dup_b(bk, g),
                "bvb": np.ascontiguousarray(ws * bv[dims]),
                "xq": f32(ws * (x[b][rows] + bo)),
                **shared,
            }
        )
    return in_maps


def kernel(x, Wq, bq, Wk, bk, Wv, bv, Wo, bo, ln_gamma, ln_beta):
    nc = _get_nc()
    in_maps = make_in_maps(x, Wq, bq, Wk, bk, Wv, bv, Wo, bo, ln_gamma, ln_beta)
    res = run_bass_kernel_spmd(nc, in_maps, core_ids=list(range(8)))
    out = np.empty((B, L, H), dtype=np.float32)
    for c in range(8):
        b, g = divmod(c, 4)
        out[b, QR * g : QR * (g + 1)] = np.asarray(
            res.results[c]["y"], dtype=np.float32
        )
    return out
